# revision 9
# baseline (speedup 1.0000x reference)
"""Trainium2 Bass kernel for nn_E29bSelectiveElmanCell.

Self-contained: accepts FULL inputs as numpy arrays, shards batch across
8 NeuronCores (data-parallel, 2 batch elements per core), runs one SPMD
Bass program, reassembles full outputs.

Returns (output_all [B,T,D], h_tape_f [B,N,D]) matching the reference.

Per-core design (all "d-major" layout: d = c*128 + p, partition p, chunk c):
  Phase 1: xz = x @ W_xz.T batched over (b, t); fp16 weights stationary,
           N=512 moving streams. b_h folded into the x_proj half. x_proj
           half -> DRAM f32, z half -> DRAM fp16.
  Phase 2: the T=512 sequential recurrence. Per step one dense PE burst
           W_cat=[W_write; W_h] @ h_new(t) (128 fp16 stationary tiles,
           N=2), attention score partials on DVE (mul+reduce over chunk
           dim), partition-sum + broadcast via one all-ones fp32 matmul
           (output replicated on 128 partitions), softmax w/o max
           subtraction (scores bounded), tape update as 3 tensor_tensor
           ops, tanh -> fp16 directly. For_i over 16 iterations x 32
           unrolled steps; xp streamed in half-chunks, h/rv histories
           accumulated into SBUF-resident fp16 tiles for phase 3.
  Phase 3: out = h_new * silu([z, rv, h_new] @ W_gate.T) batched over
           (b, t): fp16 stationary W_gate tiles, N=512 streams, silu on
           ACT, final mul on DVE.
"""

import numpy as np

B, T, D, N = 16, 512, 1024, 16
NCORES = 8
BL = B // NCORES      # 2 batch elements per core
DC = D // 128         # 8 d-chunks
GC = 3 * D // 128     # 24 gate k-chunks
U = 32                # steps per For_i iteration
NITER = T // U        # 16
SCALE = 1.0 / float(D) ** 0.5

_CACHE = {}


def _build_program(wdt_name="float16"):
    import concourse.bass as bass
    import concourse.tile as tile
    from concourse import bacc, mybir
    from contextlib import ExitStack

    dt = mybir.dt
    WDT = getattr(dt, wdt_name)
    AF = mybir.ActivationFunctionType
    OP = mybir.AluOpType
    AX = mybir.AxisListType

    nc = bacc.Bacc("TRN2", target_bir_lowering=False, debug=False,
                   num_devices=NCORES)

    # ---- DRAM I/O ----
    xT = nc.dram_tensor("xT", [128, DC, BL, T], WDT, kind="ExternalInput").ap()
    WxzT = nc.dram_tensor("WxzT", [128, DC, 2 * D], WDT, kind="ExternalInput").ap()
    WcatT = nc.dram_tensor("WcatT", [128, DC, 2 * D], WDT, kind="ExternalInput").ap()
    WgateT = nc.dram_tensor("WgateT", [128, GC, D], WDT, kind="ExternalInput").ap()
    tape0 = nc.dram_tensor("tape0", [128, DC, N, BL], dt.float32, kind="ExternalInput").ap()
    h0 = nc.dram_tensor("h0", [128, DC, BL], WDT, kind="ExternalInput").ap()
    bhT = nc.dram_tensor("bhT", [128, DC], dt.float32, kind="ExternalInput").ap()

    outT = nc.dram_tensor("outT", [128, DC, BL, T], dt.float32, kind="ExternalOutput").ap()
    tapeT_out = nc.dram_tensor("tapeT", [128, DC, N, BL], dt.float32, kind="ExternalOutput").ap()

    # DRAM scratch (xp padded by U cols so the last prefetch stays in bounds)
    xp_dram = nc.dram_tensor("xp_scratch", [128, DC, BL, T + U], dt.float32).ap()
    z_dram = nc.dram_tensor("z_scratch", [128, DC, BL, T], WDT).ap()

    def stt(ap_list):
        # shorthand to build an AP on a tile's tensor with manual dims
        t, off, dims = ap_list
        return bass.AP(t.tensor, off, dims)

    with tile.TileContext(nc, trace_sim=False) as tc:
        with ExitStack() as glob_ctx:
            # ---------- persistent pools (live across phases) ----------
            persist = glob_ctx.enter_context(tc.tile_pool(name="persist", bufs=1))
            tape = persist.tile([128, DC, N, BL], dt.float32, tag="tape")
            hist_h = persist.tile([128, DC, BL, T], WDT, tag="hist_h")
            hist_rv = persist.tile([128, DC, BL, T], WDT, tag="hist_rv")
            ones128 = persist.tile([128, 128], dt.float32, tag="ones")
            nc.gpsimd.memset(ones128[:], 1.0)
            nc.sync.dma_start(tape[:], tape0)

            # =========== Phase 1: xz = x @ W_xz.T ===========
            with ExitStack() as p1:
                p1_in = p1.enter_context(tc.tile_pool(name="p1in", bufs=1))
                p1_ps = p1.enter_context(tc.tile_pool(name="p1ps", bufs=4, space="PSUM"))
                p1_sc = p1.enter_context(tc.tile_pool(name="p1sc", bufs=4))

                xT_s = p1_in.tile([128, DC, BL, T], WDT, tag="xT_s")
                WxzT_s = p1_in.tile([128, DC, 2 * D], WDT, tag="WxzT_s")
                bh_s = p1_in.tile([128, DC], dt.float32, tag="bh_s")
                nc.sync.dma_start(xT_s[:], xT)
                nc.sync.dma_start(WxzT_s[:], WxzT)
                nc.sync.dma_start(bh_s[:], bhT)

                for jc in range(2 * DC):          # 16 output chunks of 128
                    for b in range(BL):
                        ps = p1_ps.tile([128, T], dt.float32, tag="ps")
                        for kc in range(DC):
                            nc.tensor.matmul(
                                ps[:],
                                WxzT_s[:, kc, jc * 128:(jc + 1) * 128],
                                xT_s[:, kc, b, :],
                                start=(kc == 0), stop=(kc == DC - 1),
                            )
                        if jc < DC:
                            # x_proj half: add bias, keep f32
                            xc = p1_sc.tile([128, T], dt.float32, tag="xc")
                            nc.vector.tensor_scalar_add(xc[:], ps[:], bh_s[:, jc:jc + 1])
                            nc.sync.dma_start(xp_dram[:, jc, b, 0:T], xc[:])
                        else:
                            # z half: cast to fp16 and store
                            zc = p1_sc.tile([128, T], WDT, tag="zc")
                            nc.vector.tensor_copy(zc[:], ps[:])
                            nc.sync.dma_start(z_dram[:, jc - DC, b, :], zc[:])

            # =========== Phase 2: recurrence ===========
            with ExitStack() as p2:
                p2_w = p2.enter_context(tc.tile_pool(name="p2w", bufs=1))
                p2_ps = p2.enter_context(tc.tile_pool(name="p2ps", bufs=1, space="PSUM"))
                p2_sps = p2.enter_context(tc.tile_pool(name="p2sps", bufs=2, space="PSUM"))
                p2_sc = p2.enter_context(tc.tile_pool(name="p2sc", bufs=2))
                p2_xp = p2.enter_context(tc.tile_pool(name="p2xp", bufs=1))

                Wcat_s = p2_w.tile([128, DC, 2 * D], WDT, tag="Wcat_s")
                nc.sync.dma_start(Wcat_s[:], WcatT)

                # h chunk ring: col u holds h_new(32i+u); col 31 initially h0
                hch = p2_w.tile([128, DC, BL, U], WDT, tag="hch")
                rvch = p2_w.tile([128, DC, BL, U], WDT, tag="rvch")
                nc.sync.dma_start(
                    stt((hch, U - 1, [[DC * BL * U, 128], [BL * U, DC], [U, BL]])),
                    h0)

                # xp double-buffered half-chunks [128, DC, BL, U//2]
                H = U // 2
                xp_buf = [p2_w.tile([128, DC, BL, H], dt.float32, tag=f"xpb{i}", name=f"xpb{i}")
                          for i in range(2)]
                # prologue: load first half-chunk (t = 0:16)
                nc.sync.dma_start(xp_buf[0][:], xp_dram[:, :, :, 0:H])

                # psum tiles: wh/wv ping-pong pairs + score psums
                ps_wh = [p2_ps.tile([128, DC, BL], dt.float32, tag=f"pswh{i}", name=f"pswh{i}")
                         for i in range(2)]
                ps_wv = [p2_ps.tile([128, DC, BL], dt.float32, tag=f"pswv{i}", name=f"pswv{i}")
                         for i in range(2)]

                # prologue burst: wh_pre(0) = W_h @ h0 -> ps_wh[0]
                for jc in range(DC):
                    for kc in range(DC):
                        nc.tensor.matmul(
                            ps_wh[0][:, jc, :],
                            Wcat_s[:, kc, D + jc * 128:D + (jc + 1) * 128],
                            stt((hch, kc * BL * U + (U - 1), [[DC * BL * U, 128], [U, BL]])),
                            start=(kc == 0), stop=(kc == DC - 1),
                        )

                hch_dims = [[DC * BL * U, 128], [BL * U, DC], [U, BL]]

                with tc.For_i(0, NITER, 1, hint_engines=(mybir.EngineType.PE,
                                                         mybir.EngineType.DVE)) as iv:
                    # prefetch xp for second half of this iteration
                    nc.sync.dma_start(xp_buf[1][:],
                                      xp_dram[:, :, :, bass.ds(iv * U + H, H)])
                    for u in range(U):
                        t_par = u % 2            # psum ping-pong parity
                        up = (u - 1) % U         # col of h_new(t-1)
                        # ---- B phase: read attention + activation (step t) ----
                        # h(t) = h_new(t-1) = hch col up
                        h_prev = stt((hch, up, hch_dims))  # [128, DC, BL] fp16
                        # read score partials: prod[p,(b,n),c] = tape*h
                        prod = p2_sc.tile([128, BL, N, DC], dt.float32, tag="prod")
                        nc.vector.tensor_tensor(
                            prod[:],
                            tape[:].rearrange("p c n b -> p b n c"),
                            stt((hch, up, [[DC * BL * U, 128], [U, BL], [0, N], [BL * U, DC]])),
                            OP.mult)
                        sc_s = p2_sc.tile([128, BL, N], dt.float32, tag="sc_s")
                        nc.vector.reduce_sum(sc_s[:], prod[:], AX.X)
                        # partition-sum + broadcast via ones matmul
                        ps_rs = p2_sps.tile([128, BL * N], dt.float32, tag="ps_rs")
                        nc.tensor.matmul(ps_rs[:], ones128[:],
                                         sc_s[:].rearrange("p b n -> p (b n)"),
                                         start=True, stop=True)
                        # softmax (no max subtraction; scores bounded)
                        sc_exp = p2_sc.tile([128, BL, N], dt.float32, tag="sc_exp")
                        nc.scalar.activation(
                            sc_exp[:].rearrange("p b n -> p (b n)"), ps_rs[:],
                            AF.Exp, scale=SCALE)
                        sc_den = p2_sc.tile([128, BL], dt.float32, tag="sc_den")
                        nc.vector.reduce_sum(sc_den[:], sc_exp[:], AX.X)
                        sc_rden = p2_sc.tile([128, BL], dt.float32, tag="sc_rden")
                        nc.vector.reciprocal(sc_rden[:], sc_den[:])
                        sc_ra = p2_sc.tile([128, BL, N], dt.float32, tag="sc_ra")
                        nc.vector.tensor_tensor(
                            sc_ra[:], sc_exp[:],
                            stt((sc_rden, 0, [[BL, 128], [1, BL], [0, N]])),
                            OP.mult)
                        # read_val: rv[p,c,b] = sum_n tape[p,c,n,b]*ra[b,n]
                        prod2 = p2_sc.tile([128, DC, BL, N], dt.float32, tag="prod2")
                        nc.vector.tensor_tensor(
                            prod2[:],
                            tape[:].rearrange("p c n b -> p c b n"),
                            stt((sc_ra, 0, [[BL * N, 128], [0, DC], [N, BL], [1, N]])),
                            OP.mult)
                        rv32 = p2_sc.tile([128, DC, BL], dt.float32, tag="rv32")
                        nc.vector.reduce_sum(rv32[:], prod2[:], AX.X)
                        # rv -> fp16 history column u
                        nc.vector.tensor_copy(
                            stt((rvch, u, hch_dims)), rv32[:])
                        # pre-activation: psum_wh += xp(t) ; += rv
                        xpb = xp_buf[u // H]
                        nc.vector.tensor_tensor(
                            ps_wh[t_par][:], ps_wh[t_par][:],
                            xpb[:, :, :, u % H], OP.add)
                        nc.vector.tensor_tensor(
                            ps_wh[t_par][:], ps_wh[t_par][:], rv32[:], OP.add)
                        # h_new(t) = tanh -> fp16 directly into hch col u
                        nc.scalar.activation(
                            stt((hch, u, hch_dims)), ps_wh[t_par][:], AF.Tanh)

                        # ---- PE burst t: W_cat @ h_new(t) ----
                        h_new = stt((hch, u, hch_dims))
                        for jc in range(DC):      # wv part first
                            for kc in range(DC):
                                nc.tensor.matmul(
                                    ps_wv[t_par][:, jc, :],
                                    Wcat_s[:, kc, jc * 128:(jc + 1) * 128],
                                    stt((hch, kc * BL * U + u, [[DC * BL * U, 128], [U, BL]])),
                                    start=(kc == 0), stop=(kc == DC - 1))
                        # ---- A phase: write attention + tape update (step t) ----
                        wv32 = p2_sc.tile([128, DC, BL], dt.float32, tag="wv32")
                        nc.vector.tensor_copy(wv32[:], ps_wv[t_par][:])
                        prod3 = p2_sc.tile([128, BL, N, DC], dt.float32, tag="prod")
                        nc.vector.tensor_tensor(
                            prod3[:],
                            tape[:].rearrange("p c n b -> p b n c"),
                            stt((wv32, 0, [[DC * BL, 128], [1, BL], [0, N], [BL, DC]])),
                            OP.mult)
                        sc_ws = p2_sc.tile([128, BL, N], dt.float32, tag="sc_s")
                        nc.vector.reduce_sum(sc_ws[:], prod3[:], AX.X)
                        ps_ws = p2_sps.tile([128, BL * N], dt.float32, tag="ps_rs")
                        nc.tensor.matmul(ps_ws[:], ones128[:],
                                         sc_ws[:].rearrange("p b n -> p (b n)"),
                                         start=True, stop=True)
                        sc_wexp = p2_sc.tile([128, BL, N], dt.float32, tag="sc_exp")
                        nc.scalar.activation(
                            sc_wexp[:].rearrange("p b n -> p (b n)"), ps_ws[:],
                            AF.Exp, scale=SCALE)
                        sc_wden = p2_sc.tile([128, BL], dt.float32, tag="sc_den")
                        nc.vector.reduce_sum(sc_wden[:], sc_wexp[:], AX.X)
                        sc_rwden = p2_sc.tile([128, BL], dt.float32, tag="sc_rden")
                        nc.vector.reciprocal(sc_rwden[:], sc_wden[:])
                        sc_wa = p2_sc.tile([128, BL, N], dt.float32, tag="sc_ra")
                        nc.vector.tensor_tensor(
                            sc_wa[:], sc_wexp[:],
                            stt((sc_rwden, 0, [[BL, 128], [1, BL], [0, N]])),
                            OP.mult)
                        # tape = tape + wa*(wv - tape)
                        d1 = p2_sc.tile([128, DC, N, BL], dt.float32, tag="d1")
                        nc.vector.tensor_tensor(
                            d1[:],
                            stt((wv32, 0, [[DC * BL, 128], [BL, DC], [0, N], [1, BL]])),
                            tape[:], OP.subtract)
                        d2 = p2_sc.tile([128, DC, N, BL], dt.float32, tag="d2")
                        nc.vector.tensor_tensor(
                            d2[:], d1[:],
                            stt((sc_wa, 0, [[BL * N, 128], [0, DC], [1, N], [N, BL]])),
                            OP.mult)
                        nc.vector.tensor_tensor(tape[:], tape[:], d2[:], OP.add)

                        # wh part of burst t: wh_pre(t+1) -> other psum parity
                        for jc in range(DC):
                            for kc in range(DC):
                                nc.tensor.matmul(
                                    ps_wh[(u + 1) % 2][:, jc, :],
                                    Wcat_s[:, kc, D + jc * 128:D + (jc + 1) * 128],
                                    stt((hch, kc * BL * U + u, [[DC * BL * U, 128], [U, BL]])),
                                    start=(kc == 0), stop=(kc == DC - 1))

                        # mid-iteration: history flush + xp prefetch
                        if u == H - 1:
                            nc.sync.dma_start(
                                hist_h[:, :, :, bass.ds(iv * U, H)],
                                hch[:, :, :, 0:H])
                            nc.sync.dma_start(
                                hist_rv[:, :, :, bass.ds(iv * U, H)],
                                rvch[:, :, :, 0:H])
                            # prefetch next iteration's first half
                            nc.sync.dma_start(
                                xp_buf[0][:],
                                xp_dram[:, :, :, bass.ds(iv * U + U, H)])
                        if u == U - 1:
                            nc.sync.dma_start(
                                hist_h[:, :, :, bass.ds(iv * U + H, H)],
                                hch[:, :, :, H:U])
                            nc.sync.dma_start(
                                hist_rv[:, :, :, bass.ds(iv * U + H, H)],
                                rvch[:, :, :, H:U])

                # final tape out
                nc.sync.dma_start(tapeT_out, tape[:])

            # =========== Phase 3: gated output ===========
            with ExitStack() as p3:
                p3_w = p3.enter_context(tc.tile_pool(name="p3w", bufs=1))
                p3_z = p3.enter_context(tc.tile_pool(name="p3z", bufs=3))
                p3_ps = p3.enter_context(tc.tile_pool(name="p3ps", bufs=4, space="PSUM"))
                p3_sc = p3.enter_context(tc.tile_pool(name="p3sc", bufs=3))

                Wg_s = p3_w.tile([128, GC, D], WDT, tag="Wg_s")
                nc.sync.dma_start(Wg_s[:], WgateT)

                for b in range(BL):
                    for jc in range(DC):
                        ps = p3_ps.tile([128, T], dt.float32, tag="ps")
                        for kc in range(GC):
                            if kc < DC:
                                zt = p3_z.tile([128, T], WDT, tag="zt")
                                nc.sync.dma_start(zt[:], z_dram[:, kc, b, :])
                                rhs = zt[:]
                            elif kc < 2 * DC:
                                rhs = hist_rv[:, kc - DC, b, :]
                            else:
                                rhs = hist_h[:, kc - 2 * DC, b, :]
                            nc.tensor.matmul(
                                ps[:], Wg_s[:, kc, jc * 128:(jc + 1) * 128], rhs,
                                start=(kc == 0), stop=(kc == GC - 1))
                        sg = p3_sc.tile([128, T], WDT, tag="sg")
                        nc.scalar.activation(sg[:], ps[:], AF.Silu)
                        ot = p3_sc.tile([128, T], dt.float32, tag="ot")
                        nc.vector.tensor_tensor(ot[:], sg[:], hist_h[:, jc, b, :],
                                                OP.mult)
                        nc.sync.dma_start(outT[:, jc, b, :], ot[:])

    nc.compile()
    return nc


def _get_program():
    if "nc" not in _CACHE:
        _CACHE["nc"] = _build_program()
    return _CACHE["nc"]


def _dmaj(a):
    """Convert the LAST axis (a multiple of 128) to d-major layout.

    Input a [..., K*128]; returns array indexed [128, K, ...] where
    result[p, k, ...] = a[..., k*128 + p]."""
    k = a.shape[-1] // 128
    dc = a.reshape(*a.shape[:-1], k, 128)           # [..., k, p]
    nd = dc.ndim
    return np.ascontiguousarray(np.transpose(dc, (nd - 1, nd - 2) + tuple(range(nd - 2))))


def kernel(x, h_tape_init, h_work_init, W_h, W_xz, b_h, W_write, W_gate):
    from concourse import bass_utils

    x = np.asarray(x, dtype=np.float32)
    h_tape_init = np.asarray(h_tape_init, dtype=np.float32)
    h_work_init = np.asarray(h_work_init, dtype=np.float32)
    W_h = np.asarray(W_h, dtype=np.float32)
    W_xz = np.asarray(W_xz, dtype=np.float32)
    b_h = np.asarray(b_h, dtype=np.float32)
    W_write = np.asarray(W_write, dtype=np.float32)
    W_gate = np.asarray(W_gate, dtype=np.float32)

    WDT = np.float16

    # Shared (replicated) weight layouts.
    # WxzT[p, c, j'] = W_xz[j', c*128+p]: last axis of W_xz is d -> _dmaj,
    # leading axis j' moves to the free position.
    WxzT = _dmaj(W_xz).astype(WDT)                            # [128, DC, 2048]
    WcatT = _dmaj(np.vstack([W_write, W_h])).astype(WDT)      # [128, DC, 2048]
    # WgateT[p, kc, j] = W_gate[j, kc*128+p]: last axis of W_gate is k (3D)
    WgateT = _dmaj(W_gate).astype(WDT)                        # [128, GC, 1024]
    bhT = _dmaj(b_h)                                          # [128, DC]

    core_ids = list(range(NCORES))
    in_maps = []
    for c in core_ids:
        bs = slice(c * BL, (c + 1) * BL)
        xs = x[bs]                                            # [BL, T, D]
        in_maps.append({
            "xT": np.ascontiguousarray(_dmaj(xs)).astype(WDT),  # [128, DC, BL, T]
            "WxzT": WxzT,
            "WcatT": WcatT,
            "WgateT": WgateT,
            "tape0": np.ascontiguousarray(_dmaj(h_tape_init[bs])),  # [128,DC,BL? ...]
            "h0": np.ascontiguousarray(_dmaj(h_work_init[bs])).astype(WDT),
            "bhT": np.ascontiguousarray(bhT),
        })
    # fix tape0 axis order: _dmaj(h_tape_init[bs]) gives [p, c, b, n]; want [p, c, n, b]
    for m in in_maps:
        m["tape0"] = np.ascontiguousarray(m["tape0"].transpose(0, 1, 3, 2))

    nc = _get_program()
    _CACHE["last_in_maps"] = in_maps
    res = bass_utils.run_bass_kernel_spmd(nc, in_maps, core_ids)

    out = np.empty((B, T, D), dtype=np.float32)
    tape_f = np.empty((B, N, D), dtype=np.float32)
    for c in core_ids:
        r = res.results[c]
        oT = r["outT"]          # [128, DC, BL, T]
        tT = r["tapeT"]         # [128, DC, N, BL]
        # out[b, t, c*128+p] = oT[p, c, b, t]
        out[c * BL:(c + 1) * BL] = (
            oT.transpose(2, 3, 1, 0).reshape(BL, T, D))
        tape_f[c * BL:(c + 1) * BL] = (
            tT.transpose(3, 2, 1, 0).reshape(BL, N, D))
    return out, tape_f


if __name__ == "__main__":
    rng = np.random.default_rng(0)
    s = 1.0 / np.sqrt(D)
    ins = {
        "x": rng.standard_normal((B, T, D), dtype=np.float32),
        "h_tape_init": rng.standard_normal((B, N, D), dtype=np.float32) * s,
        "h_work_init": rng.standard_normal((B, D), dtype=np.float32) * s,
        "W_h": rng.standard_normal((D, D), dtype=np.float32) * s,
        "W_xz": rng.standard_normal((2 * D, D), dtype=np.float32) * s,
        "b_h": np.zeros(D, dtype=np.float32),
        "W_write": rng.standard_normal((D, D), dtype=np.float32) * s,
        "W_gate": rng.standard_normal((D, 3 * D), dtype=np.float32) * s,
    }
    o, tf = kernel(**ins)
    print("ran", o.shape, tf.shape)
